# revision 22
# baseline (speedup 1.0000x reference)
"""Multi-branch BatchNorm2d (16 branches sharing one batch-stat reduction).

Computation (reference):
    mean/var over (B,H,W) per channel of x[32,64,32,32], then for each of
    N=16 branches: out[:, n*64:(n+1)*64] = gamma[n,c]*xhat + beta[n,c],
    giving out[32, 1024, 32, 32].

Strategy (8 NeuronCores, branch-parallel, no collectives):
  - x is replicated: every core reads the full 8 MiB x and computes the
    (B,H,W) mean/var locally. A 1 KB all-reduce would instead allow a
    batch-sharded read (1 MiB/core), but the ncfw collective measures
    70-80 us/call on this setup - far more than the 20 us of extra read.
    With no cross-core dependency, each core's span is independent of
    dispatch stagger.
  - SBUF layout [128, 32, 512]: partition p = c*2 + h0 (h0 = H half),
    free (b, (h1, w)). x is loaded in decreasing batch chunks so the
    per-batch bn_stats pipeline drains right behind the last DMA.
  - The (c,0)/(c,1) partition pair is summed via two small SBUF-to-SBUF
    DMAs (no HBM bounce), then mean = S/32768, inv = rsqrt(var+eps) are
    folded with gamma/beta into per-(branch,channel) scale A = gamma*inv,
    bias B = beta - mean*A.
  - Each core computes N/8 = 2 branches: 16 fused tensor_scalar ops
    (out = x*A + B) + 16 x 1 MiB DMA stores = 16 MiB of output writes per
    core, the HBM roofline for this memory-bound problem.
"""

import numpy as np

import concourse.bacc as bacc
import concourse.bass as bass
import concourse.tile as tile
from concourse import mybir
from concourse.bass_utils import run_bass_kernel_spmd

B, C, H, W = 32, 64, 32, 32
N = 16
NCORES = 8
NL = N // NCORES           # 2 branches per core
H2 = H // 2                # 16
FB = H2 * W                # 512 free elems per batch per partition
NTOT = float(B * H * W)    # 32768 elements reduced per channel
EPS = 1e-5
F32 = mybir.dt.float32

# Load chunks (in batches): flat 1 MiB chunks keep the read stream at line
# rate; a small final chunk keeps the stats tail behind the last DMA short.
CHUNKS = [8, 8, 8, 4, 3, 1]
assert sum(CHUNKS) == B

WG = 4                     # batches per write group

_NC_CACHE = {}


def _build():
    # Bacc (not raw Bass): its generate_event_semaphores pass legalizes
    # instructions down to <=1 sync-wait each (walrus TS encodings cannot
    # carry more).
    nc = bacc.Bacc("TRN2", num_devices=NCORES, target_bir_lowering=False,
                   debug=False)
    x = nc.dram_tensor("x", [B, C, H, W], F32, kind="ExternalInput")
    gn = nc.dram_tensor("gn", [2 * C, NL], F32, kind="ExternalInput")
    bn = nc.dram_tensor("bn", [2 * C, NL], F32, kind="ExternalInput")
    out = nc.dram_tensor("out", [B, NL * C, H, W], F32, kind="ExternalOutput")

    # [128, 32, 512]: partition (c h0), free (b, h1*w)
    x_re = x.ap().rearrange("b c (h0 h1) w -> (c h0) b (h1 w)", h0=2)
    # [2, 128, 32, 512]
    out_re = out.ap().rearrange("b (n c) (h0 h1) w -> n (c h0) b (h1 w)",
                                n=NL, h0=2)

    with tile.TileContext(nc) as tc:
        with (
            tc.tile_pool(name="xin", bufs=1) as xin,
            tc.tile_pool(name="consts", bufs=1) as consts,
            tc.tile_pool(name="small", bufs=1) as small,
            tc.tile_pool(name="outs", bufs=10) as outs,
        ):
            sbuf_eps = small.tile([128, 1], F32)
            nc.vector.memset(sbuf_eps, EPS)

            # Per-(c,h0) gamma/beta for this core's branches, pre-transposed
            # on host: [128, 2].
            g_sb = consts.tile([2 * C, NL], F32)
            b_sb = consts.tile([2 * C, NL], F32)
            nc.gpsimd.dma_start(out=g_sb, in_=gn.ap())
            nc.gpsimd.dma_start(out=b_sb, in_=bn.ap())

            # Full x, loaded in batch chunks. Per chunk, two accumulating
            # passes pipeline behind the DMA in parallel: ACT computes the
            # chunk sum (Copy + accum_out; Copy needs no LUT table), DVE the
            # chunk sum-of-squares (tensor_tensor_reduce x*x).
            nchunk = len(CHUNKS)
            x_sb = xin.tile([2 * C, B, FB], F32)
            junk_s = small.tile([128, max(CHUNKS) * FB], F32, tag="junk_s")
            junk_q = small.tile([128, max(CHUNKS) * FB], F32, tag="junk_q")
            s_cols = small.tile([128, nchunk], F32)
            q_cols = small.tile([128, nchunk], F32)
            b0 = 0
            for ci, nb in enumerate(CHUNKS):
                nc.sync.dma_start(out=x_sb[:, b0:b0 + nb, :],
                                  in_=x_re[:, b0:b0 + nb, :])
                xc = x_sb[:, b0:b0 + nb, :].rearrange("p b f -> p (b f)")
                nc.vector.tensor_scalar(
                    out=junk_s[:, 0:nb * FB], in0=xc,
                    scalar1=1.0, scalar2=0.0, op0=mybir.AluOpType.mult,
                    op1=mybir.AluOpType.add,
                    accum_out=s_cols[:, ci:ci + 1])
                nc.scalar.activation(
                    out=junk_q[:, 0:nb * FB], in_=xc,
                    func=mybir.ActivationFunctionType.Square,
                    accum_out=q_cols[:, ci:ci + 1])
                b0 += nb

            # (S, Q) per partition (per H-half), then pair-combine via the
            # DVE 32-way partition permute: swapped[p] = part[p^1].
            part = small.tile([128, 2], F32)
            nc.vector.reduce_sum(out=part[:, 0:1], in_=s_cols,
                                 axis=mybir.AxisListType.X)
            nc.vector.reduce_sum(out=part[:, 1:2], in_=q_cols,
                                 axis=mybir.AxisListType.X)
            swapped = small.tile([128, 2], F32)
            pairswap = [i ^ 1 for i in range(32)]
            nc.vector.stream_shuffle(out=swapped, in_=part[:, :],
                                     mask=pairswap)
            stt = small.tile([128, 2], F32)  # (S_tot, Q_tot) per channel
            nc.vector.tensor_add(out=stt, in0=part[:, :], in1=swapped)

            mean = small.tile([128, 1], F32)
            nc.vector.tensor_scalar_mul(out=mean, in0=stt[:, 0:1],
                                        scalar1=1.0 / NTOT)
            ex2t = small.tile([128, 1], F32)
            nc.vector.tensor_scalar_mul(out=ex2t, in0=stt[:, 1:2],
                                        scalar1=1.0 / NTOT)
            msq2 = small.tile([128, 1], F32)
            nc.vector.tensor_mul(out=msq2, in0=mean, in1=mean)
            var = small.tile([128, 1], F32)
            nc.vector.tensor_sub(out=var, in0=ex2t, in1=msq2)
            sd = small.tile([128, 1], F32)
            nc.scalar.activation(out=sd, in_=var,
                                 func=mybir.ActivationFunctionType.Sqrt,
                                 bias=sbuf_eps[:, :])
            inv = small.tile([128, 1], F32)
            nc.vector.reciprocal(out=inv, in_=sd)

            # A = gamma*inv ; Bc = beta - mean*A.
            a_sb = consts.tile([128, NL], F32)
            nc.vector.tensor_scalar_mul(out=a_sb, in0=g_sb, scalar1=inv)
            ma = consts.tile([128, NL], F32)
            nc.vector.tensor_scalar_mul(out=ma, in0=a_sb, scalar1=mean)
            bc_sb = consts.tile([128, NL], F32)
            nc.vector.tensor_sub(out=bc_sb, in0=b_sb, in1=ma)

            # Main loop: fused multiply-add + 1 MiB store per (branch, group).
            for j in range(NL):
                for g in range(B // WG):
                    o = outs.tile([128, WG * FB], F32)
                    xg = x_sb[:, g * WG:(g + 1) * WG, :].rearrange(
                        "p b f -> p (b f)")
                    nc.vector.tensor_scalar(
                        out=o, in0=xg,
                        scalar1=a_sb[:, j:j + 1], scalar2=bc_sb[:, j:j + 1],
                        op0=mybir.AluOpType.mult, op1=mybir.AluOpType.add,
                    )
                    nc.sync.dma_start(
                        out=out_re[j][:, g * WG:(g + 1) * WG, :], in_=o)
    # Run Bacc's compile pipeline (event-sem legalization, register
    # allocation); the PJRT execute path serializes without finalizing.
    nc.finalize()
    return nc


def _get_nc():
    if "nc" not in _NC_CACHE:
        _NC_CACHE["nc"] = _build()
    return _NC_CACHE["nc"]


def _run(inputs, **kwargs):
    x = np.ascontiguousarray(np.asarray(inputs["x"], dtype=np.float32))
    gamma = np.asarray(inputs["gamma"], dtype=np.float32)
    beta = np.asarray(inputs["beta"], dtype=np.float32)
    g128 = np.ascontiguousarray(np.repeat(gamma.T, 2, axis=0))  # [128, 16]
    b128 = np.ascontiguousarray(np.repeat(beta.T, 2, axis=0))
    in_maps = [
        {"x": x,
         "gn": np.ascontiguousarray(g128[:, i * NL:(i + 1) * NL]),
         "bn": np.ascontiguousarray(b128[:, i * NL:(i + 1) * NL])}
        for i in range(NCORES)
    ]
    nc = _get_nc()
    res = run_bass_kernel_spmd(nc, in_maps, core_ids=list(range(NCORES)), **kwargs)
    # Core i computed branches [i*NL, (i+1)*NL) -> channel block of NL*C.
    full = np.concatenate([r["out"] for r in res.results], axis=1)
    return full, res


def kernel(**inputs):
    full, _ = _run(inputs)
    return full


# revision 23
# speedup vs baseline: 1.0391x; 1.0391x over previous
"""Multi-branch BatchNorm2d (16 branches sharing one batch-stat reduction).

Computation (reference):
    mean/var over (B,H,W) per channel of x[32,64,32,32], then for each of
    N=16 branches: out[:, n*64:(n+1)*64] = gamma[n,c]*xhat + beta[n,c],
    giving out[32, 1024, 32, 32].

Strategy (8 NeuronCores, branch-parallel, no collectives):
  - x is replicated: every core reads the full 8 MiB x and computes the
    (B,H,W) mean/var locally. A 1 KB all-reduce would instead allow a
    batch-sharded read (1 MiB/core), but the ncfw collective measures
    70-80 us/call on this setup - far more than the 20 us of extra read.
    With no cross-core dependency, each core's span is independent of
    dispatch stagger.
  - SBUF layout [128, 32, 512]: partition p = c*2 + h0 (h0 = H half),
    free (b, (h1, w)). x is loaded in decreasing batch chunks so the
    per-batch bn_stats pipeline drains right behind the last DMA.
  - The (c,0)/(c,1) partition pair is summed via two small SBUF-to-SBUF
    DMAs (no HBM bounce), then mean = S/32768, inv = rsqrt(var+eps) are
    folded with gamma/beta into per-(branch,channel) scale A = gamma*inv,
    bias B = beta - mean*A.
  - Each core computes N/8 = 2 branches: 16 fused tensor_scalar ops
    (out = x*A + B) + 16 x 1 MiB DMA stores = 16 MiB of output writes per
    core, the HBM roofline for this memory-bound problem.
"""

import numpy as np

import concourse.bacc as bacc
import concourse.bass as bass
import concourse.tile as tile
from concourse import mybir
from concourse.bass_utils import run_bass_kernel_spmd

B, C, H, W = 32, 64, 32, 32
N = 16
NCORES = 8
NL = N // NCORES           # 2 branches per core
H2 = H // 2                # 16
FB = H2 * W                # 512 free elems per batch per partition
NTOT = float(B * H * W)    # 32768 elements reduced per channel
EPS = 1e-5
F32 = mybir.dt.float32

# Load chunks (in batches): flat 1 MiB chunks keep the read stream at line
# rate; a small final chunk keeps the stats tail behind the last DMA short.
CHUNKS = [4, 4, 4, 4, 4, 4, 4, 3, 1]
assert sum(CHUNKS) == B

WG = 4                     # batches per write group

_NC_CACHE = {}


def _build():
    # Bacc (not raw Bass): its generate_event_semaphores pass legalizes
    # instructions down to <=1 sync-wait each (walrus TS encodings cannot
    # carry more).
    nc = bacc.Bacc("TRN2", num_devices=NCORES, target_bir_lowering=False,
                   debug=False)
    x = nc.dram_tensor("x", [B, C, H, W], F32, kind="ExternalInput")
    gn = nc.dram_tensor("gn", [2 * C, NL], F32, kind="ExternalInput")
    bn = nc.dram_tensor("bn", [2 * C, NL], F32, kind="ExternalInput")
    out = nc.dram_tensor("out", [B, NL * C, H, W], F32, kind="ExternalOutput")

    # [128, 32, 512]: partition (c h0), free (b, h1*w)
    x_re = x.ap().rearrange("b c (h0 h1) w -> (c h0) b (h1 w)", h0=2)
    # [2, 128, 32, 512]
    out_re = out.ap().rearrange("b (n c) (h0 h1) w -> n (c h0) b (h1 w)",
                                n=NL, h0=2)

    with tile.TileContext(nc) as tc:
        with (
            tc.tile_pool(name="xin", bufs=1) as xin,
            tc.tile_pool(name="consts", bufs=1) as consts,
            tc.tile_pool(name="small", bufs=1) as small,
            tc.tile_pool(name="outs", bufs=10) as outs,
        ):
            sbuf_eps = small.tile([128, 1], F32)
            nc.vector.memset(sbuf_eps, EPS)

            # Per-(c,h0) gamma/beta for this core's branches, pre-transposed
            # on host: [128, 2].
            g_sb = consts.tile([2 * C, NL], F32)
            b_sb = consts.tile([2 * C, NL], F32)
            nc.gpsimd.dma_start(out=g_sb, in_=gn.ap())
            nc.gpsimd.dma_start(out=b_sb, in_=bn.ap())

            # Full x, loaded in batch chunks. Per chunk, two accumulating
            # passes pipeline behind the DMA in parallel: ACT computes the
            # chunk sum (Copy + accum_out; Copy needs no LUT table), DVE the
            # chunk sum-of-squares (tensor_tensor_reduce x*x).
            nchunk = len(CHUNKS)
            x_sb = xin.tile([2 * C, B, FB], F32)
            junk_s = small.tile([128, max(CHUNKS) * FB], F32, tag="junk_s")
            junk_q = small.tile([128, max(CHUNKS) * FB], F32, tag="junk_q")
            s_cols = small.tile([128, nchunk], F32)
            q_cols = small.tile([128, nchunk], F32)
            b0 = 0
            for ci, nb in enumerate(CHUNKS):
                nc.sync.dma_start(out=x_sb[:, b0:b0 + nb, :],
                                  in_=x_re[:, b0:b0 + nb, :])
                xc = x_sb[:, b0:b0 + nb, :].rearrange("p b f -> p (b f)")
                nc.vector.tensor_scalar(
                    out=junk_s[:, 0:nb * FB], in0=xc,
                    scalar1=1.0, scalar2=0.0, op0=mybir.AluOpType.mult,
                    op1=mybir.AluOpType.add,
                    accum_out=s_cols[:, ci:ci + 1])
                nc.scalar.activation(
                    out=junk_q[:, 0:nb * FB], in_=xc,
                    func=mybir.ActivationFunctionType.Square,
                    accum_out=q_cols[:, ci:ci + 1])
                b0 += nb

            # (S, Q) per partition (per H-half), then pair-combine via the
            # DVE 32-way partition permute: swapped[p] = part[p^1].
            part = small.tile([128, 2], F32)
            nc.vector.reduce_sum(out=part[:, 0:1], in_=s_cols,
                                 axis=mybir.AxisListType.X)
            nc.vector.reduce_sum(out=part[:, 1:2], in_=q_cols,
                                 axis=mybir.AxisListType.X)
            swapped = small.tile([128, 2], F32)
            pairswap = [i ^ 1 for i in range(32)]
            nc.vector.stream_shuffle(out=swapped, in_=part[:, :],
                                     mask=pairswap)
            stt = small.tile([128, 2], F32)  # (S_tot, Q_tot) per channel
            nc.vector.tensor_add(out=stt, in0=part[:, :], in1=swapped)

            mean = small.tile([128, 1], F32)
            nc.vector.tensor_scalar_mul(out=mean, in0=stt[:, 0:1],
                                        scalar1=1.0 / NTOT)
            ex2t = small.tile([128, 1], F32)
            nc.vector.tensor_scalar_mul(out=ex2t, in0=stt[:, 1:2],
                                        scalar1=1.0 / NTOT)
            msq2 = small.tile([128, 1], F32)
            nc.vector.tensor_mul(out=msq2, in0=mean, in1=mean)
            var = small.tile([128, 1], F32)
            nc.vector.tensor_sub(out=var, in0=ex2t, in1=msq2)
            sd = small.tile([128, 1], F32)
            nc.scalar.activation(out=sd, in_=var,
                                 func=mybir.ActivationFunctionType.Sqrt,
                                 bias=sbuf_eps[:, :])
            inv = small.tile([128, 1], F32)
            nc.vector.reciprocal(out=inv, in_=sd)

            # A = gamma*inv ; Bc = beta - mean*A.
            a_sb = consts.tile([128, NL], F32)
            nc.vector.tensor_scalar_mul(out=a_sb, in0=g_sb, scalar1=inv)
            ma = consts.tile([128, NL], F32)
            nc.vector.tensor_scalar_mul(out=ma, in0=a_sb, scalar1=mean)
            bc_sb = consts.tile([128, NL], F32)
            nc.vector.tensor_sub(out=bc_sb, in0=b_sb, in1=ma)

            # Main loop: fused multiply-add + 1 MiB store per (branch, group).
            for j in range(NL):
                for g in range(B // WG):
                    o = outs.tile([128, WG * FB], F32)
                    xg = x_sb[:, g * WG:(g + 1) * WG, :].rearrange(
                        "p b f -> p (b f)")
                    nc.vector.tensor_scalar(
                        out=o, in0=xg,
                        scalar1=a_sb[:, j:j + 1], scalar2=bc_sb[:, j:j + 1],
                        op0=mybir.AluOpType.mult, op1=mybir.AluOpType.add,
                    )
                    nc.sync.dma_start(
                        out=out_re[j][:, g * WG:(g + 1) * WG, :], in_=o)
    # Run Bacc's compile pipeline (event-sem legalization, register
    # allocation); the PJRT execute path serializes without finalizing.
    nc.finalize()
    return nc


def _get_nc():
    if "nc" not in _NC_CACHE:
        _NC_CACHE["nc"] = _build()
    return _NC_CACHE["nc"]


def _run(inputs, **kwargs):
    x = np.ascontiguousarray(np.asarray(inputs["x"], dtype=np.float32))
    gamma = np.asarray(inputs["gamma"], dtype=np.float32)
    beta = np.asarray(inputs["beta"], dtype=np.float32)
    g128 = np.ascontiguousarray(np.repeat(gamma.T, 2, axis=0))  # [128, 16]
    b128 = np.ascontiguousarray(np.repeat(beta.T, 2, axis=0))
    in_maps = [
        {"x": x,
         "gn": np.ascontiguousarray(g128[:, i * NL:(i + 1) * NL]),
         "bn": np.ascontiguousarray(b128[:, i * NL:(i + 1) * NL])}
        for i in range(NCORES)
    ]
    nc = _get_nc()
    res = run_bass_kernel_spmd(nc, in_maps, core_ids=list(range(NCORES)), **kwargs)
    # Core i computed branches [i*NL, (i+1)*NL) -> channel block of NL*C.
    full = np.concatenate([r["out"] for r in res.results], axis=1)
    return full, res


def kernel(**inputs):
    full, _ = _run(inputs)
    return full


# revision 24
# speedup vs baseline: 1.1309x; 1.0884x over previous
"""Multi-branch BatchNorm2d (16 branches sharing one batch-stat reduction).

Computation (reference):
    mean/var over (B,H,W) per channel of x[32,64,32,32], then for each of
    N=16 branches: out[:, n*64:(n+1)*64] = gamma[n,c]*xhat + beta[n,c],
    giving out[32, 1024, 32, 32].

Strategy (8 NeuronCores, branch-parallel, no collectives):
  - x is replicated: every core reads the full 8 MiB x and computes the
    (B,H,W) mean/var locally. A 1 KB all-reduce would instead allow a
    batch-sharded read (1 MiB/core), but the ncfw collective measures
    70-80 us/call on this setup - far more than the 20 us of extra read.
    With no cross-core dependency, each core's span is independent of
    dispatch stagger.
  - SBUF layout [128, 32, 512]: partition p = c*2 + h0 (h0 = H half),
    free (b, (h1, w)). x is loaded in decreasing batch chunks so the
    per-batch bn_stats pipeline drains right behind the last DMA.
  - The (c,0)/(c,1) partition pair is summed via two small SBUF-to-SBUF
    DMAs (no HBM bounce), then mean = S/32768, inv = rsqrt(var+eps) are
    folded with gamma/beta into per-(branch,channel) scale A = gamma*inv,
    bias B = beta - mean*A.
  - Each core computes N/8 = 2 branches: 16 fused tensor_scalar ops
    (out = x*A + B) + 16 x 1 MiB DMA stores = 16 MiB of output writes per
    core, the HBM roofline for this memory-bound problem.
"""

import numpy as np

import concourse.bacc as bacc
import concourse.bass as bass
import concourse.tile as tile
from concourse import mybir
from concourse.bass_utils import run_bass_kernel_spmd

B, C, H, W = 32, 64, 32, 32
N = 16
NCORES = 8
NL = N // NCORES           # 2 branches per core
H2 = H // 2                # 16
FB = H2 * W                # 512 free elems per batch per partition
NTOT = float(B * H * W)    # 32768 elements reduced per channel
EPS = 1e-5
F32 = mybir.dt.float32

# Load chunks (in batches): flat 1 MiB chunks keep the read stream at line
# rate; a small final chunk keeps the stats tail behind the last DMA short.
CHUNKS = [4, 4, 4, 4, 4, 4, 4, 3, 1]
assert sum(CHUNKS) == B

WG = 4                     # batches per write group

_NC_CACHE = {}


def _build():
    # Bacc (not raw Bass): its generate_event_semaphores pass legalizes
    # instructions down to <=1 sync-wait each (walrus TS encodings cannot
    # carry more).
    nc = bacc.Bacc("TRN2", num_devices=NCORES, target_bir_lowering=False,
                   debug=False)
    x = nc.dram_tensor("x", [B, C, H, W], F32, kind="ExternalInput")
    gn = nc.dram_tensor("gn", [2 * C, NL], F32, kind="ExternalInput")
    bn = nc.dram_tensor("bn", [2 * C, NL], F32, kind="ExternalInput")
    out = nc.dram_tensor("out", [B, NL * C, H, W], F32, kind="ExternalOutput")

    # [128, 32, 512]: partition (c h0), free (b, h1*w)
    x_re = x.ap().rearrange("b c (h0 h1) w -> (c h0) b (h1 w)", h0=2)
    # [2, 128, 32, 512]
    out_re = out.ap().rearrange("b (n c) (h0 h1) w -> n (c h0) b (h1 w)",
                                n=NL, h0=2)

    with tile.TileContext(nc) as tc:
        with (
            tc.tile_pool(name="xin", bufs=1) as xin,
            tc.tile_pool(name="consts", bufs=1) as consts,
            tc.tile_pool(name="small", bufs=1) as small,
            tc.tile_pool(name="outs", bufs=10) as outs,
        ):
            sbuf_eps = small.tile([128, 1], F32)
            nc.vector.memset(sbuf_eps, EPS)

            # Per-(c,h0) gamma/beta for this core's branches, pre-transposed
            # on host: [128, 2].
            g_sb = consts.tile([2 * C, NL], F32)
            b_sb = consts.tile([2 * C, NL], F32)
            nc.gpsimd.dma_start(out=g_sb, in_=gn.ap())
            nc.gpsimd.dma_start(out=b_sb, in_=bn.ap())

            # Full x, loaded in batch chunks. Per chunk, two accumulating
            # passes pipeline behind the DMA in parallel: ACT computes the
            # chunk sum (Copy + accum_out; Copy needs no LUT table), DVE the
            # chunk sum-of-squares (tensor_tensor_reduce x*x).
            nchunk = len(CHUNKS)
            x_sb = xin.tile([2 * C, B, FB], F32)
            junk_s = small.tile([128, max(CHUNKS) * FB], F32, tag="junk_s")
            junk_q = small.tile([128, max(CHUNKS) * FB], F32, tag="junk_q")
            sq_cols = small.tile([128, 2, nchunk], F32)
            b0 = 0
            for ci, nb in enumerate(CHUNKS):
                nc.sync.dma_start(out=x_sb[:, b0:b0 + nb, :],
                                  in_=x_re[:, b0:b0 + nb, :])
                xc = x_sb[:, b0:b0 + nb, :].rearrange("p b f -> p (b f)")
                nc.vector.tensor_scalar(
                    out=junk_s[:, 0:nb * FB], in0=xc,
                    scalar1=1.0, scalar2=0.0, op0=mybir.AluOpType.mult,
                    op1=mybir.AluOpType.add,
                    accum_out=sq_cols[:, 0, ci:ci + 1].rearrange(
                        "p a -> p (a)"))
                nc.scalar.activation(
                    out=junk_q[:, 0:nb * FB], in_=xc,
                    func=mybir.ActivationFunctionType.Square,
                    accum_out=sq_cols[:, 1, ci:ci + 1].rearrange(
                        "p a -> p (a)"))
                b0 += nb

            # (S, Q) per partition (per H-half), then pair-combine via the
            # DVE 32-way partition permute: swapped[p] = part[p^1].
            part = small.tile([128, 2], F32)
            nc.vector.reduce_sum(out=part, in_=sq_cols,
                                 axis=mybir.AxisListType.X)
            swapped = small.tile([128, 2], F32)
            pairswap = [i ^ 1 for i in range(32)]
            nc.vector.stream_shuffle(out=swapped, in_=part[:, :],
                                     mask=pairswap)
            stt = small.tile([128, 2], F32)  # (S_tot, Q_tot) per channel
            nc.vector.tensor_add(out=stt, in0=part[:, :], in1=swapped)

            # (mean, E[x^2]) in one op; var = ex2 - mean^2 via the negated
            # mean so it fuses into a single scalar_tensor_tensor.
            me = small.tile([128, 2], F32)
            nc.vector.tensor_scalar_mul(out=me, in0=stt, scalar1=1.0 / NTOT)
            mean = me[:, 0:1]
            nmean = small.tile([128, 1], F32)
            nc.vector.tensor_scalar_mul(out=nmean, in0=mean, scalar1=-1.0)
            var = small.tile([128, 1], F32)
            nc.vector.scalar_tensor_tensor(
                out=var, in0=nmean, scalar=mean, in1=me[:, 1:2],
                op0=mybir.AluOpType.mult, op1=mybir.AluOpType.add)
            sd = small.tile([128, 1], F32)
            nc.scalar.activation(out=sd, in_=var,
                                 func=mybir.ActivationFunctionType.Sqrt,
                                 bias=sbuf_eps[:, :])
            inv = small.tile([128, 1], F32)
            nc.vector.reciprocal(out=inv, in_=sd)

            # A = gamma*inv ; Bc = beta + nmean*A.
            a_sb = consts.tile([128, NL], F32)
            nc.vector.tensor_scalar_mul(out=a_sb, in0=g_sb, scalar1=inv)
            bc_sb = consts.tile([128, NL], F32)
            nc.vector.scalar_tensor_tensor(
                out=bc_sb, in0=a_sb, scalar=nmean, in1=b_sb,
                op0=mybir.AluOpType.mult, op1=mybir.AluOpType.add)

            # Main loop: fused multiply-add + store per (branch, group).
            # Group sizes ramp up so the first store DMA issues as soon as
            # possible after the fold.
            groups = []
            gb = 0
            for wg in [1, 1, 2] + [WG] * ((B - 4) // WG):
                groups.append((gb, wg))
                gb += wg
            assert gb == B
            for j in range(NL):
                for (gb0, wg) in groups:
                    o = outs.tile([128, WG * FB], F32, tag="o")
                    xg = x_sb[:, gb0:gb0 + wg, :].rearrange("p b f -> p (b f)")
                    nc.vector.tensor_scalar(
                        out=o[:, 0:wg * FB], in0=xg,
                        scalar1=a_sb[:, j:j + 1], scalar2=bc_sb[:, j:j + 1],
                        op0=mybir.AluOpType.mult, op1=mybir.AluOpType.add,
                    )
                    nc.sync.dma_start(
                        out=out_re[j][:, gb0:gb0 + wg, :], in_=o[:, 0:wg * FB])
    # Run Bacc's compile pipeline (event-sem legalization, register
    # allocation); the PJRT execute path serializes without finalizing.
    nc.finalize()
    return nc


def _get_nc():
    if "nc" not in _NC_CACHE:
        _NC_CACHE["nc"] = _build()
    return _NC_CACHE["nc"]


def _run(inputs, **kwargs):
    x = np.ascontiguousarray(np.asarray(inputs["x"], dtype=np.float32))
    gamma = np.asarray(inputs["gamma"], dtype=np.float32)
    beta = np.asarray(inputs["beta"], dtype=np.float32)
    g128 = np.ascontiguousarray(np.repeat(gamma.T, 2, axis=0))  # [128, 16]
    b128 = np.ascontiguousarray(np.repeat(beta.T, 2, axis=0))
    in_maps = [
        {"x": x,
         "gn": np.ascontiguousarray(g128[:, i * NL:(i + 1) * NL]),
         "bn": np.ascontiguousarray(b128[:, i * NL:(i + 1) * NL])}
        for i in range(NCORES)
    ]
    nc = _get_nc()
    res = run_bass_kernel_spmd(nc, in_maps, core_ids=list(range(NCORES)), **kwargs)
    # Core i computed branches [i*NL, (i+1)*NL) -> channel block of NL*C.
    full = np.concatenate([r["out"] for r in res.results], axis=1)
    return full, res


def kernel(**inputs):
    full, _ = _run(inputs)
    return full
